# revision 2
# baseline (speedup 1.0000x reference)
"""Fused cross-entropy loss over a 100k item vocabulary on 8 Trainium2 cores.

Math (matches the reference):
    logits = hidden_flat @ item_emb.T          # [1024, 100000]
    nll[r] = log(sum_v exp(logits[r, v])) - logits[r, label[r]]
    loss   = sum(w * nll) / sum(w)             # w = active-token mask

Sharding: the vocab dim is split across the 8 cores (12500 each). Every core
computes partial row-sums S_c[r] = sum_{v in shard} exp(logits[r, v]) with
bf16 matmuls (fp32 PSUM accumulate) + a fused ACT exp/row-sum, then one tiny
AllReduce (4 KB) combines the denominators. Label logits are computed exactly
in fp32 (row-wise dot products) redundantly on every core, so no second
collective is needed. The final masked mean is computed on-device.
"""
import sys

if "/opt/trn_rl_repo" not in sys.path:
    sys.path.insert(0, "/opt/trn_rl_repo")

import numpy as np

import concourse.bass as bass
import concourse.bacc as bacc
import concourse.tile as tile
import concourse.mybir as mybir
from concourse import bass_utils

N_CORES = 8
B, L, D = 8, 128, 768
V = 100000
VS = V // N_CORES            # vocab shard per core
T = B * L                    # 1024 token rows (last row per batch is masked out)
KC = D // 128                # contraction chunks
NUM_USERS = 10000
LABEL_OFFSET = 151669 + NUM_USERS

BF16 = mybir.dt.bfloat16
F32 = mybir.dt.float32
NP_BF16 = mybir.dt.np(BF16)

# vocab chunks per core: 6x2048 + 1x212 (psum tile = 4 banks = 2048 fp32)
CHUNK_W = 2048
CHUNKS = [(j * CHUNK_W, CHUNK_W) for j in range(VS // CHUNK_W)]
if VS % CHUNK_W:
    CHUNKS.append((VS - VS % CHUNK_W, VS % CHUNK_W))

_prog_cache = {}


def build_program(repeat: int = 1):
    if repeat in _prog_cache:
        return _prog_cache[repeat]
    nc = bacc.Bacc(
        "TRN2",
        target_bir_lowering=False,
        debug=False,
        enable_asserts=True,
        num_devices=N_CORES,
    )
    hT = nc.dram_tensor("hT", [D, T], BF16, kind="ExternalInput")
    eT = nc.dram_tensor("eT", [D, VS], BF16, kind="ExternalInput")
    hpb = nc.dram_tensor("hpb", [128, B * D], F32, kind="ExternalInput")
    gpb = nc.dram_tensor("gpb", [128, B * D], F32, kind="ExternalInput")
    wpb = nc.dram_tensor("wpb", [128, B], F32, kind="ExternalInput")
    loss = nc.dram_tensor("loss", [1, 1], F32, kind="ExternalOutput")

    add = mybir.AluOpType.add
    mult = mybir.AluOpType.mult
    AF = mybir.ActivationFunctionType
    AX = mybir.AxisListType

    with tile.TileContext(nc) as tc:
        with (
            tc.tile_pool(name="const", bufs=1) as cpool,
            tc.tile_pool(name="rhs", bufs=2) as rpool,
            tc.tile_pool(name="escr", bufs=1) as epool,
            tc.tile_pool(name="psum", bufs=2, space="PSUM") as ppool,
            tc.tile_pool(name="dram", bufs=1, space="DRAM") as dpool,
        ):
            # resident tensors
            ht_sb = cpool.tile([128, KC, T], BF16)
            nc.sync.dma_start(ht_sb[:], hT.ap().rearrange("(k p) t -> p k t", p=128))
            hpb_sb = cpool.tile([128, B * D], F32)
            nc.sync.dma_start(hpb_sb[:], hpb.ap())
            gpb_sb = cpool.tile([128, B * D], F32)
            nc.sync.dma_start(gpb_sb[:], gpb.ap())
            wpb_sb = cpool.tile([128, B], F32)
            nc.sync.dma_start(wpb_sb[:], wpb.ap())
            ones_sb = cpool.tile([128, 1], F32)
            nc.vector.memset(ones_sb[:], 1.0)

            # exact fp32 label logits: dot[p, i] = <hidden[i,p,:], emb[label]>
            dot_sb = cpool.tile([128, B], F32)
            tscr = cpool.tile([128, D], F32)
            for i in range(B):
                nc.vector.tensor_mul(
                    tscr[:],
                    hpb_sb[:, i * D : (i + 1) * D],
                    gpb_sb[:, i * D : (i + 1) * D],
                )
                nc.vector.tensor_reduce(
                    out=dot_sb[:, i : i + 1], in_=tscr[:], axis=AX.X, op=add
                )

            n2 = cpool.tile([128, 2], F32)
            nc.vector.tensor_reduce(
                out=n2[:, 1:2], in_=wpb_sb[:], axis=AX.X, op=add
            )

            # main loop: partial exp row-sums over this core's vocab shard
            r_sb = cpool.tile([128, B, len(CHUNKS)], F32)
            eT_r = eT.ap().rearrange("(k p) v -> p k v", p=128)

            def main_loop(_iv=None):
                for ci, (jstart, W) in enumerate(CHUNKS):
                    rt = rpool.tile([128, KC, CHUNK_W], BF16, tag="rt", name=f"rt{ci}")
                    nc.sync.dma_start(
                        rt[:, :, :W], eT_r[:, :, jstart : jstart + W]
                    )
                    nbank = (W + 511) // 512
                    for i in range(B):
                        pt = ppool.tile([128, CHUNK_W], F32, tag="pt", name=f"pt{ci}_{i}")
                        for k in range(KC):
                            for b in range(nbank):
                                s = 512 * b
                                e = min(W, s + 512)
                                nc.tensor.matmul(
                                    pt[:, s:e],
                                    lhsT=ht_sb[:, k, i * 128 : (i + 1) * 128],
                                    rhs=rt[:, k, s:e],
                                    start=(k == 0),
                                    stop=(k == KC - 1),
                                )
                        et = epool.tile([128, CHUNK_W], F32, tag="et", name=f"et{ci}_{i}")
                        nc.scalar.activation(
                            et[:, :W],
                            pt[:, :W],
                            AF.Exp,
                            accum_out=r_sb[:, i, ci : ci + 1],
                        )

            if repeat == 1:
                main_loop()
            else:
                with tc.For_i(0, repeat, 1) as iv:
                    main_loop(iv)

            s_sb = cpool.tile([128, B], F32)
            nc.vector.tensor_reduce(out=s_sb[:], in_=r_sb[:], axis=AX.X, op=add)

            # AllReduce the partial softmax denominators (4 KB)
            cc_in = dpool.tile([128, B], F32)
            cc_out = dpool.tile([128, B], F32, addr_space="Shared")
            nc.sync.dma_start(cc_in[:], s_sb[:])
            nc.gpsimd.collective_compute(
                "AllReduce",
                add,
                replica_groups=[list(range(N_CORES))],
                ins=[cc_in.opt()],
                outs=[cc_out.opt()],
            )
            stot = cpool.tile([128, B], F32)
            nc.sync.dma_start(stot[:], cc_out[:])

            # loss = sum(w * (ln(S) - dot)) / sum(w)
            lt = cpool.tile([128, B], F32)
            nc.scalar.activation(lt[:], stot[:], AF.Ln)
            u = cpool.tile([128, B], F32)
            nc.vector.tensor_sub(u[:], lt[:], dot_sb[:])
            nc.vector.tensor_mul(u[:], u[:], wpb_sb[:])
            nc.vector.tensor_reduce(out=n2[:, 0:1], in_=u[:], axis=AX.X, op=add)
            ps2 = ppool.tile([1, 2], F32, tag="pt", name="ps2")
            nc.tensor.matmul(ps2[:], lhsT=ones_sb[:], rhs=n2[:], start=True, stop=True)
            inv = cpool.tile([1, 1], F32)
            nc.vector.reciprocal(inv[:], ps2[:, 1:2])
            res = cpool.tile([1, 1], F32)
            nc.vector.tensor_mul(res[:], ps2[:, 0:1], inv[:])
            nc.sync.dma_start(loss.ap(), res[:])

    nc.compile()
    _prog_cache[repeat] = nc
    return nc


def prepare_in_maps(hidden, item_emb, labels_main, attention_mask, prompt_length):
    hidden = np.asarray(hidden, dtype=np.float32).reshape(B, L, D)
    item_emb = np.asarray(item_emb, dtype=np.float32).reshape(V, D)
    labels_main = np.asarray(labels_main).reshape(B, L)
    attention_mask = np.asarray(attention_mask)
    pl = int(prompt_length)

    active = attention_mask[:, pl + 1 :] == 1  # [B, L-1]
    assert active.shape == (B, L - 1), active.shape

    hT = np.ascontiguousarray(
        hidden.reshape(T, D).T.astype(NP_BF16)
    )  # [D, T] bf16
    hpb = np.ascontiguousarray(
        hidden.transpose(1, 0, 2).reshape(128, B * D)
    )  # [p, i*D+d]

    lab = np.zeros((128, B), dtype=np.int64)
    lab[: L - 1, :] = np.clip(
        labels_main[:, 1:].T - LABEL_OFFSET, 0, V - 1
    )
    gpb = np.ascontiguousarray(
        item_emb[lab.reshape(-1)].reshape(128, B * D)
    )

    w = np.zeros((128, B), dtype=np.float32)
    w[: L - 1, :] = active.T.astype(np.float32)

    eT = np.ascontiguousarray(item_emb.astype(NP_BF16).T)  # [D, V] bf16

    in_maps = []
    for c in range(N_CORES):
        in_maps.append(
            {
                "hT": hT,
                "eT": np.ascontiguousarray(eT[:, c * VS : (c + 1) * VS]),
                "hpb": hpb,
                "gpb": gpb,
                "wpb": w,
            }
        )
    return in_maps


def kernel(hidden, item_emb, labels_main, attention_mask, prompt_length):
    in_maps = prepare_in_maps(
        hidden, item_emb, labels_main, attention_mask, prompt_length
    )
    nc = build_program()
    res = bass_utils.run_bass_kernel_spmd(
        nc, in_maps, core_ids=list(range(N_CORES))
    )
    return np.float32(res.results[0]["loss"][0, 0])
